# revision 9
# baseline (speedup 1.0000x reference)
"""Trainium2 Bass kernel for CapsuleFC EM-routing (nn_CapsuleFC_73246372266532).

Strategy (8 NeuronCores, one chip):
  - Shard W and votes over Nin (512 -> 64 per core). W is read exactly once
    across the fleet (33.5 MB/core). votes_local [B=32, 64, 32, 64] f32
    (16.8 MB) stays RESIDENT in SBUF for all routing iterations.
  - votes layout "b-major": partition p = b + 32*(n%4), free = (g=n//4, m, d).
    Produced by 4-way col-packed PE matmuls (tile_position=(0,32j)),
    contraction over Din=64.
  - Routing per iteration: logits via DVE mult + segmented reduce over d;
    softmax over m fully local; aggregation via DVE mult + PE "ones" matmul
    (contracts the 4 partition groups and accumulates the 16 n-groups in
    PSUM). Cross-core reduction of the [32,32,64] aggregate via AllReduce
    (4 total, ~12us each).
  - route_class_emb falls out of the last iteration's aggregation multiply
    (t2 = wq*votes) and is DMA'd out per n-group.

The kernel() entry point takes FULL inputs, shards on host, runs SPMD on
cores 0-7, and reassembles FULL outputs.
"""

import numpy as np

import concourse.bass as bass
import concourse.mybir as mybir
import concourse.tile as tile
from concourse import bacc
from concourse.bass_utils import run_bass_kernel_spmd

F32 = mybir.dt.float32

B = 32       # batch
NIN = 512    # full input capsules
NC = 8       # cores
NL = NIN // NC   # local input capsules = 64
A = 64       # Din
M = 32       # Nout
D = 64       # Dout
G = NL // 4  # n-groups of 4 -> 16
MD = M * D   # 2048
SCALE = 1.0 / np.sqrt(D)  # 0.125


def _build(num_iter: int, use_cc: bool = True):
    """Build the per-core SPMD program (identical on all cores)."""
    nc = bacc.Bacc(
        "TRN2", target_bir_lowering=False, debug=False, num_devices=NC
    )

    # ---- kernel I/O (per-core) ----
    poseT = nc.dram_tensor("poseT", [A, NL, B], F32, kind="ExternalInput").ap()
    w = nc.dram_tensor("w", [NL, A, MD], F32, kind="ExternalInput").ap()
    actpn = nc.dram_tensor("actpn", [128, G], F32, kind="ExternalInput").ap()
    ones4b = nc.dram_tensor("ones4b", [128, 128], F32, kind="ExternalInput").ap()
    qk_out = nc.dram_tensor("qk_out", [128, G, M], F32, kind="ExternalOutput").ap()
    emb_out = nc.dram_tensor("emb_out", [G, 128, MD], F32, kind="ExternalOutput").ap()
    ncv_out = nc.dram_tensor("ncv_out", [B, MD], F32, kind="ExternalOutput").ap()
    nact_out = nc.dram_tensor("nact_out", [B, M], F32, kind="ExternalOutput").ap()

    rg = [list(range(NC))]

    with tile.TileContext(nc) as tc:
        with (
            tc.tile_pool(name="singles", bufs=1) as singles,
            tc.tile_pool(name="wpool", bufs=3) as wpool,
            tc.tile_pool(name="scratch", bufs=2) as scratch,
            tc.tile_pool(name="ncvb", bufs=1) as ncvbp,
            tc.tile_pool(name="prodp", bufs=2, space="PSUM") as prodp,
            tc.tile_pool(name="accp", bufs=1, space="PSUM") as accp,
            tc.tile_pool(name="dram", bufs=2, space="DRAM") as dram,
        ):
            # ---- resident SBUF tiles ----
            votes = singles.tile([128, G, 4, 512], F32)   # 128 KB/partition
            poseT_sb = singles.tile([A, NL, B], F32)      # 8 KB
            logits = singles.tile([128, G, M], F32)       # 2 KB
            qk = singles.tile([128, G, M], F32)
            wq = singles.tile([128, G, M], F32)
            acc0 = singles.tile([128, MD], F32)           # ncv0 accumulator
            ncv_part = singles.tile([B, 4, 512], F32)     # AR staging
            smalls = singles.tile([128, 256], F32)        # packed tiny tiles
            ones_sb = smalls[:, 0:128]
            actpn_sb = smalls[:, 128 : 128 + G]
            ssum = smalls[:, 144 : 144 + G]
            recip = smalls[:, 160 : 160 + G]
            s2 = smalls[0:B, 176 : 176 + M]
            nact_sb = smalls[0:B, 208 : 208 + M]

            # ---- one-time loads ----
            nc.sync.dma_start(out=poseT_sb[:], in_=poseT)
            nc.sync.dma_start(out=ones_sb, in_=ones4b)
            nc.sync.dma_start(out=actpn_sb, in_=actpn)

            # ================= production: votes = pose @ W ================
            # per n: out[b, md] = sum_a poseT[a, n, b] * W[n, a, md]
            # 4 n's col-packed into partition blocks 32j.
            for g in range(G):
                for h in range(2):  # md half-chunks: c = 2h, 2h+1
                    ph = prodp.tile([128, 2, 512], F32, tag="ph")
                    for j in range(4):
                        n = 4 * g + j
                        wt = wpool.tile([A, 1024], F32, tag="wt")
                        nc.sync.dma_start(
                            out=wt[:], in_=w[n][:, 1024 * h : 1024 * h + 1024]
                        )
                        for cc in range(2):
                            nc.tensor.matmul(
                                ph[32 * j : 32 * j + 32, cc, :],
                                poseT_sb[:, n, :],
                                wt[:, 512 * cc : 512 * cc + 512],
                                start=True,
                                stop=True,
                                tile_position=(0, 32 * j),
                            )
                    for cc in range(2):
                        nc.scalar.copy(
                            out=votes[:, g, 2 * h + cc, :], in_=ph[:, cc, :]
                        )
                # ncv0 partial accumulation on (otherwise idle) DVE
                vg = votes[:, g].rearrange("p c x -> p (c x)")
                if g == 0:
                    nc.vector.tensor_copy(out=acc0[:], in_=vg)
                else:
                    nc.vector.tensor_add(acc0[:], acc0[:], vg)

            # ncv0: fold the 4 partition groups with the ones matmul.
            # NOTE: acc0 is sum over local n of votes (missing the 1/Nout);
            # compensated by exp-scale on iteration 1.
            def ones_fold(rhs_flat, psum_tile, start, stop):
                for c in range(4):
                    nc.tensor.matmul(
                        psum_tile[:, c, :],
                        ones_sb,
                        rhs_flat[:, 512 * c : 512 * c + 512],
                        start=start,
                        stop=stop,
                        skip_group_check=True,
                    )

            accps = accp.tile([128, 4, 512], F32, tag="acc")
            ones_fold(acc0[:], accps, True, True)

            def all_reduce(psum_tile):
                arin_t = dram.tile([B, 4, 512], F32, tag="arin")
                arout_t = dram.tile([B, 4, 512], F32, tag="arout")
                nc.scalar.copy(out=ncv_part[:], in_=psum_tile[0:B])
                nc.sync.dma_start(out=arin_t[:], in_=ncv_part[:])
                if use_cc:
                    nc.gpsimd.collective_compute(
                        "AllReduce",
                        mybir.AluOpType.add,
                        replica_groups=rg,
                        ins=[arin_t[:].opt()],
                        outs=[arout_t[:].opt()],
                    )
                else:
                    nc.sync.dma_start(out=arout_t[:], in_=arin_t[:])
                return arout_t

            def bcast_ncv(arout_t):
                # replicate [32, 2048] DRAM -> [128, 2048] SBUF (4x over j)
                nb = ncvbp.tile([128, M, D], F32, tag="nb")
                src = arout_t[:].rearrange("b c x -> b (c x)").rearrange(
                    "b (m d) -> b m d", m=M
                )
                for r in range(4):
                    nc.sync.dma_start(out=nb[32 * r : 32 * r + 32], in_=src)
                return nb

            arout_t = all_reduce(accps)
            ncv_b = bcast_ncv(arout_t)

            # ===================== routing iterations ======================
            for k in range(1, num_iter + 1):
                last = k == num_iter
                # iteration 1 uses the unnormalized ncv0 (missing 1/Nout)
                exp_scale = float(SCALE / M) if k == 1 else float(SCALE)

                # --- logits ---
                for g in range(G):
                    t = scratch.tile([128, M, D], F32, tag="t")
                    nc.vector.tensor_mul(t[:], votes[:, g].rearrange("p c x -> p (c x)").rearrange("p (m d) -> p m d", m=M), ncv_b[:])
                    nc.vector.tensor_reduce(
                        out=logits[:, g, :],
                        in_=t[:],
                        axis=mybir.AxisListType.X,
                        op=mybir.AluOpType.add,
                    )

                # --- softmax over m (local: all m on-core) ---
                nc.scalar.activation(
                    out=qk[:],
                    in_=logits[:],
                    func=mybir.ActivationFunctionType.Exp,
                    scale=exp_scale,
                )
                nc.vector.tensor_reduce(
                    out=ssum,
                    in_=qk[:],
                    axis=mybir.AxisListType.X,
                    op=mybir.AluOpType.add,
                )
                nc.vector.reciprocal(recip, ssum)
                nc.vector.tensor_mul(
                    qk[:], qk[:], recip.unsqueeze(2).broadcast_to([128, G, M])
                )
                nc.vector.tensor_mul(
                    wq[:], qk[:], actpn_sb.unsqueeze(2).broadcast_to([128, G, M])
                )
                if last:
                    nc.sync.dma_start(out=qk_out, in_=qk[:])

                # --- aggregation ---
                accps_k = accp.tile([128, 4, 512], F32, tag="acc")
                for g in range(G):
                    t2 = scratch.tile([128, M, D], F32, tag="t")
                    nc.vector.tensor_mul(
                        t2[:],
                        votes[:, g].rearrange("p c x -> p (c x)").rearrange("p (m d) -> p m d", m=M),
                        wq[:, g, :].unsqueeze(2).broadcast_to([128, M, D]),
                    )
                    ones_fold(
                        t2[:].rearrange("p m d -> p (m d)"),
                        accps_k,
                        g == 0,
                        g == G - 1,
                    )
                    if last:
                        nc.sync.dma_start(
                            out=emb_out[g],
                            in_=t2[:].rearrange("p m d -> p (m d)"),
                        )

                arout_t = all_reduce(accps_k)
                if not last:
                    ncv_b = bcast_ncv(arout_t)

            # ===================== finals =================================
            nc.sync.dma_start(
                out=ncv_out, in_=arout_t[:].rearrange("b c x -> b (c x)")
            )
            nc.sync.dma_start(out=ncv_part[:], in_=arout_t[:])
            ncv_v = ncv_part[:].rearrange("b c x -> b (c x)").rearrange(
                "b (m d) -> b m d", m=M
            )
            sq_v = acc0[0:B].rearrange("b (m d) -> b m d", m=M)
            nc.vector.tensor_mul(sq_v, ncv_v, ncv_v)
            nc.vector.tensor_reduce(
                out=s2,
                in_=sq_v,
                axis=mybir.AxisListType.X,
                op=mybir.AluOpType.add,
            )
            nc.scalar.activation(
                out=nact_sb, in_=s2, func=mybir.ActivationFunctionType.Sqrt
            )
            nc.sync.dma_start(out=nact_out, in_=nact_sb)

    nc.compile()
    return nc


_PROG_CACHE = {}


def _get_prog(num_iter: int):
    if num_iter not in _PROG_CACHE:
        _PROG_CACHE[num_iter] = _build(num_iter)
    return _PROG_CACHE[num_iter]


def _host_inputs(input_pose, current_act, W):
    pose = np.ascontiguousarray(np.asarray(input_pose, dtype=np.float32))
    act = np.ascontiguousarray(np.asarray(current_act, dtype=np.float32))
    Wf = np.asarray(W, dtype=np.float32)
    ones4b = np.tile(np.eye(B, dtype=np.float32), (4, 4))  # [128,128]
    in_maps = []
    for c in range(NC):
        sl = slice(NL * c, NL * (c + 1))
        poseT = np.ascontiguousarray(pose[:, sl, :].transpose(2, 1, 0))  # [A,NL,B]
        w_c = np.ascontiguousarray(Wf[sl].reshape(NL, A, MD))
        act_c = act[:, sl]  # [B, NL]
        # actpn[b + 32*j, g] = act_c[b, 4g + j]
        actpn = np.ascontiguousarray(
            act_c.reshape(B, G, 4).transpose(2, 0, 1).reshape(128, G)
        )
        in_maps.append(
            {"poseT": poseT, "w": w_c, "actpn": actpn, "ones4b": ones4b}
        )
    return in_maps


def _assemble(results):
    # qk: [128, G, M] per core -> [B, NIN, M]
    qk_parts = []
    emb_parts = []
    for c in range(NC):
        qk_c = results[c]["qk_out"].reshape(4, B, G, M)       # [j,b,g,m]
        qk_parts.append(qk_c.transpose(1, 2, 0, 3).reshape(B, NL, M))
        emb_c = results[c]["emb_out"].reshape(G, 4, B, M, D)  # [g,j,b,m,d]
        emb_parts.append(emb_c.transpose(2, 0, 1, 3, 4).reshape(B, NL, M, D))
    qk_full = np.concatenate(qk_parts, axis=1)
    emb_full = np.concatenate(emb_parts, axis=1)
    ncv = results[0]["ncv_out"].reshape(B, M, D)
    nact = results[0]["nact_out"].reshape(B, M)
    return ncv, nact, qk_full, emb_full


def run_sharded(input_pose, current_act, W, num_iter, **spmd_kwargs):
    """Run the SPMD kernel; returns (outputs_tuple, BassKernelResults)."""
    iters = max(1, int(num_iter))
    nc = _get_prog(iters)
    in_maps = _host_inputs(input_pose, current_act, W)
    res = run_bass_kernel_spmd(nc, in_maps, core_ids=list(range(NC)), **spmd_kwargs)
    return _assemble(res.results), res


def kernel(input_pose, current_act, W, num_iter):
    outs, _ = run_sharded(input_pose, current_act, W, num_iter)
    return outs
